# revision 22
# baseline (speedup 1.0000x reference)
"""Trainium2 Bass kernel for Felsenstein pruning on a perfect binary tree
(nn_BaseLikelihoodModel: batched expm over edges + level-synchronous sweep).

Key algorithmic idea: every edge matrix is expm(t_e * R) for ONE shared
16x16 rate matrix R = Q - diag(growth).  A real block-eigendecomposition
R = W M W^-1 (computed host-side in f64, O(16^3)) turns the per-edge expm
into per-edge exp/sin factors plus shared 16x16 matmuls:

    expm(t R) v = W * B(t) * (W^-1 v)
    B(t) rows:   real eig k:    e^{a t} u_k
                 pair (k,k+1):  e^{a t}(cos(bt) u_k + sin(bt) u_{k+1})
                                e^{a t}(cos(bt) u_{k+1} - sin(bt) u_k)

The level sweep runs in probability space with per-node rescaling every 4
levels (accumulated per-node log-scales), mathematically identical to the
reference's log-space logsumexp recursion.

Sharding: 8 contiguous subtrees of 4096 leaves (one per core).  Within a
core, 8 partition-blocks of 16 states each hold 8 sub-subtrees of 512
leaves, giving full 128-partition utilization for levels 1..9.  Levels
10..12 run on a single 16-partition block.  One tiny AllGather (8 x 32 f32)
shares the per-core subtree roots + log-scales; levels 13..16 are computed
redundantly on every core.

Numerics: sweep matmuls run in bf16 (inputs) with f32 PSUM accumulation;
the per-node normalization is self-consistent (the same computed sum is
both divided out and logged), so quantization there cancels.  Edge factors
carry a constant boost e^BOOST to keep normalization sums inside the HW Ln
table's [2^-64, 2^64] domain; the exact total (N-1)*BOOST is subtracted at
the end.
"""
import math
import numpy as np
import ml_dtypes

import concourse.bass as bass
import concourse.mybir as mybir
import concourse.tile as tile
from concourse.bass_utils import run_bass_kernel_spmd

F32 = np.float32
BF16 = ml_dtypes.bfloat16
S = 16
L = 32768
N = 2 * L
NCORES = 8
NBLK = 8
LPC = L // NCORES          # 4096 leaves per core
LPB = LPC // NBLK          # 512 leaves per block
# block-sweep children widths per block for h=1..9
BLK_W = [LPB >> hc for hc in range(9)]            # 512..2
BLK_OFF = np.concatenate([[0], np.cumsum(BLK_W)])  # offsets into 1022
BLK_TOTAL = int(BLK_OFF[-1])                       # 1022
# top-edge factors live in block 0 of the same tiles, columns TOPO..TOPO+29
# layout: [lvl9 x8][lvl10 x4][lvl11 x2][lvl12 x8][lvl13 x4][lvl14 x2][lvl15 x1]
TOPO = BLK_TOTAL                                   # 1022
T_ALL = 1056                                       # padded total columns
CHUNKS = [(0, 512), (512, 512), (1024, 32)]
NORM_LEVELS = (4, 8)                               # + level 12 pre-gather

# global level offsets (node ids are level-contiguous, leaves first)
OFFS = [0]
for _h in range(1, 16):
    OFFS.append(OFFS[-1] + (L >> (_h - 1)))
ROOT = N - 1  # 65535; its only child is OFFS[15] = 65534

# Constant per-edge boost e^BOOST folded into the exp factors so the
# per-node normalization sums stay near 1.0 -- the HW Ln table only covers
# [2^-64, 2^64] (it clamps outside, which the simulator's exact np.log
# does not model).  The exact total, CORR, is subtracted at the end.
BOOST = 1.7
CORR = float(np.float64(N - 1) * np.float64(np.float32(BOOST)))
AGW = 32  # AllGather payload per core (16 state values + logscale + pad)


def _real_eig(R):
    """Real block eigendecomposition R = Wr @ M @ inv(Wr) with M block
    diagonal ([[a, b], [-b, a]] blocks for conjugate pairs).  Returns
    f64 Wr, Winv, a[k], bsig[k] (signed imag: +b row k, -b row k+1),
    swap[k] (pair permutation, identity on real rows)."""
    ev, V = np.linalg.eig(R)
    used = np.zeros(S, bool)
    order = np.argsort(-ev.real)
    cols = []
    for i in order:
        if used[i]:
            continue
        if abs(ev[i].imag) < 1e-12:
            cols.append(("real", i))
            used[i] = True
        else:
            j = None
            for i2 in order:
                if not used[i2] and i2 != i and abs(ev[i2] - ev[i].conj()) < 1e-8:
                    j = i2
                    break
            assert j is not None, "unpaired complex eigenvalue"
            ip = i if ev[i].imag > 0 else j
            cols.append(("pair", ip))
            used[i] = used[j] = True
    Wr = np.zeros((S, S))
    a = np.zeros(S)
    bsig = np.zeros(S)
    swap = np.arange(S)
    k = 0
    for c in cols:
        if c[0] == "real":
            i = c[1]
            Wr[:, k] = V[:, i].real
            a[k] = ev[i].real
            k += 1
        else:
            ip = c[1]
            lam = ev[ip]
            Wr[:, k] = V[:, ip].real
            Wr[:, k + 1] = V[:, ip].imag
            a[k] = a[k + 1] = lam.real
            bsig[k] = lam.imag
            bsig[k + 1] = -lam.imag
            swap[k] = k + 1
            swap[k + 1] = k
            k += 2
    assert k == S
    # column-normalize for conditioning; pairs share a scale to keep the
    # block structure valid
    scales = np.ones(S)
    kk = 0
    while kk < S:
        if swap[kk] == kk:
            scales[kk] = np.linalg.norm(Wr[:, kk])
            kk += 1
        else:
            s = math.sqrt(np.linalg.norm(Wr[:, kk]) * np.linalg.norm(Wr[:, kk + 1]))
            scales[kk] = scales[kk + 1] = s
            kk += 2
    Wr = Wr / scales[None, :]
    Winv = np.linalg.inv(Wr)
    return Wr, Winv, a, bsig, swap


def _split_multi_waits(nc):
    """Walrus codegen allows only ONE sync-wait slot per engine instruction
    (NEURON_ISA_TPB_EVENTS has a single wait field).  Tile's scheduler can
    attach several; move the extras onto prepended same-engine NoOps, which
    execute in queue order and therefore stall identically."""
    skip = (mybir.InstAllEngineBarrier, mybir.InstBranchHint,
            mybir.InstCompareAndBranch, mybir.InstUnconditionalBranch,
            mybir.InstIndirectBranch)
    for fn in nc.m.functions:
        for blk in fn.blocks:
            out = []
            for inst in blk.instructions:
                si = inst.sync_info
                if (si is not None and si.on_wait and len(si.on_wait) > 1
                        and not isinstance(inst, skip)):
                    waits = list(si.on_wait)
                    for i, w in enumerate(waits[:-1]):
                        nop = mybir.InstNoOp(
                            name=f"{inst.name}-wait{i}", ins=[], outs=[])
                        nop.engine = inst.engine
                        nop.sync_info = mybir.SyncInfo(
                            on_wait=[w], on_update=[])
                        out.append(nop)
                    inst.sync_info = mybir.SyncInfo(
                        on_wait=[waits[-1]], on_update=list(si.on_update or []))
                out.append(inst)
            blk.instructions = out


def build_nc(split_waits=True):
    f32 = mybir.dt.float32
    bf16 = mybir.dt.bfloat16
    AF = mybir.ActivationFunctionType
    OP = mybir.AluOpType
    nc = bass.Bass()

    # per-core inputs
    a_leaf = nc.dram_tensor("a_leaf", [128, 512], f32, kind="ExternalInput")
    t_blk = nc.dram_tensor("t_blk", [8, T_ALL], f32, kind="ExternalInput")
    # shared constants (bf16 for the sweep matmuls)
    winvT = nc.dram_tensor("winvT_bd", [128, 128], bf16, kind="ExternalInput")
    wswapT = nc.dram_tensor("wswapT_bd", [128, 128], bf16, kind="ExternalInput")
    wT = nc.dram_tensor("wT_bd", [128, 128], bf16, kind="ExternalInput")
    onesbd = nc.dram_tensor("ones_bd", [128, 128], bf16, kind="ExternalInput")
    onesc = nc.dram_tensor("onesc", [128, 8], bf16, kind="ExternalInput")
    itile = nc.dram_tensor("itile", [128, 16], bf16, kind="ExternalInput")
    ident = nc.dram_tensor("ident", [128, 128], f32, kind="ExternalInput")
    expd = nc.dram_tensor("expd", [8, 128], f32, kind="ExternalInput")
    avec = nc.dram_tensor("avec", [128, 1], f32, kind="ExternalInput")
    bvec = nc.dram_tensor("bvec", [128, 1], f32, kind="ExternalInput")
    onesf = nc.dram_tensor("onesf", [1, 16], f32, kind="ExternalInput")
    gcol = nc.dram_tensor("gcol", [128, 1], f32, kind="ExternalInput")
    id8f = nc.dram_tensor("id8f", [8, 8], f32, kind="ExternalInput")
    out = nc.dram_tensor("out", [16, 1], f32, kind="ExternalOutput")
    # collective bounce buffers (internal DRAM)
    agin = nc.dram_tensor("agin", [AGW, 1], f32)
    agout = nc.dram_tensor("agout", [NCORES * AGW, 1], f32, addr_space="Shared")

    PI2 = float(np.pi / 2)

    with tile.TileContext(nc) as tc:
        with (
            tc.tile_pool(name="const", bufs=1) as cp,
            tc.tile_pool(name="sb", bufs=2) as sb,
            tc.tile_pool(name="big", bufs=1) as bigp,
            tc.tile_pool(name="psT", bufs=2, space="PSUM") as psT,
            tc.tile_pool(name="psU", bufs=1, space="PSUM") as psU,
            tc.tile_pool(name="psUs", bufs=1, space="PSUM") as psUs,
            tc.tile_pool(name="psYe", bufs=2, space="PSUM") as psYe,
            tc.tile_pool(name="psYo", bufs=2, space="PSUM") as psYo,
        ):
            # ---- constants into SBUF (order matters: early ones feed the
            # first compute)
            def cload(dram, shape, dt, tag):
                t = cp.tile(shape, dt, tag=tag)
                nc.sync.dma_start(t[:], dram[:, :])
                return t

            sTb = cp.tile([8, T_ALL], f32, tag="tblk")
            nc.sync.dma_start(sTb[:], t_blk[:, :])
            c_expd = cload(expd, [8, 128], f32, "expd")
            c_avec = cload(avec, [128, 1], f32, "avec")
            c_bvec = cload(bvec, [128, 1], f32, "bvec")
            sA = bigp.tile([128, 512], f32, tag="aleaf")
            nc.sync.dma_start(sA[:], a_leaf[:, :])
            c_ident = cload(ident, [128, 128], f32, "ident")
            c_winvT = cload(winvT, [128, 128], bf16, "winvT")
            c_wswapT = cload(wswapT, [128, 128], bf16, "wswapT")
            c_wT = cload(wT, [128, 128], bf16, "wT")
            c_onesbd = cload(onesbd, [128, 128], bf16, "onesbd")
            c_onesc = cload(onesc, [128, 8], bf16, "onesc")
            c_itile = cload(itile, [128, 16], bf16, "itile")
            c_onesf = cload(onesf, [1, 16], f32, "onesf")
            c_id8f = cload(id8f, [8, 8], f32, "id8f")
            c_gcol = cload(gcol, [128, 1], f32, "gcol")
            cPI2 = cp.tile([128, 1], f32, tag="pi2")
            nc.vector.memset(cPI2[:], PI2)
            cBOOST = cp.tile([128, 1], f32, tag="boost")
            nc.vector.memset(cBOOST[:], float(BOOST))

            # ---- phase 1: leaf one-hots (exp of log one-hots) + transpose
            # into blocked layout X[16u+c, 128k+j] = onehot[leaf 512u+128k+j][c]
            sAe = bigp.tile([128, 512], f32, tag="aexp")
            nc.scalar.activation(sAe[:], sA[:], AF.Exp)
            sX = bigp.tile([128, 512], bf16, tag="V0")
            for k in range(4):
                pT = psT.tile([128, 128], f32, tag="T")
                nc.tensor.transpose(pT[:], sAe[:, 128 * k:128 * (k + 1)],
                                    c_ident[:])
                nc.vector.tensor_copy(sX[:, 128 * k:128 * (k + 1)], pT[:])

            # ---- phase 2: edge factors EC = e^{at+BOOST}cos(bt),
            # ES = e^{at+BOOST}sin(bt) for all edges (block sweep cols
            # 0..1021, top edges in block 0 cols 1022..1050).
            # T128 = t broadcast down each 16-partition block (one matmul),
            # then AT/BT via per-partition scalar mults straight to SBUF;
            # activations batched by table set (exp first, then sin).
            AT = bigp.tile([128, T_ALL], f32, tag="AT")
            BT = bigp.tile([128, T_ALL], f32, tag="BT")
            for lo, wch in CHUNKS:
                pT128 = psT.tile([128, wch], f32, tag="T")
                nc.tensor.matmul(pT128[:], c_expd[:], sTb[:, lo:lo + wch],
                                 start=True, stop=True)
                nc.vector.tensor_scalar_mul(AT[:, lo:lo + wch], pT128[:],
                                            c_avec[:, 0:1])
                nc.vector.tensor_scalar_mul(BT[:, lo:lo + wch], pT128[:],
                                            c_bvec[:, 0:1])
            # exp/sins and the EC/ES multiplies chunked so the columns
            # level 1 needs are ready before the tail is processed; exps all
            # precede sins so each ACT table set loads exactly once
            # ---- PE queue-observer preamble: one tiny matmul per DMA'd
            # matmul operand so later matmuls never need two DMA-queue waits
            # (walrus allows a single sync wait per PE instruction).
            obs_bf = (c_winvT, c_wswapT, c_wT, c_onesbd, c_onesc,
                      c_itile)
            pobs = psYe.tile([1, 1], f32, tag="Ye")
            for i, o in enumerate(obs_bf):
                nc.tensor.matmul(pobs[:], o[0:1, 0:1], o[0:1, 0:1],
                                 start=(i == 0), stop=(i == len(obs_bf) - 1))
            obs_f = (c_ident, c_expd, sTb, c_onesf, c_id8f)
            pobs2 = psYo.tile([1, 1], f32, tag="Yo")
            for i, o in enumerate(obs_f):
                nc.tensor.matmul(pobs2[:], o[0:1, 0:1], o[0:1, 0:1],
                                 start=(i == 0), stop=(i == len(obs_f) - 1))

            sE = bigp.tile([128, T_ALL], f32, tag="sE")
            for lo, wch in ((0, 512), (512, T_ALL - 512)):
                nc.scalar.activation(sE[:, lo:lo + wch], AT[:, lo:lo + wch],
                                     AF.Exp, bias=cBOOST[:, 0:1])
            sC = bigp.tile([128, T_ALL], f32, tag="sC")
            sSn = bigp.tile([128, T_ALL], f32, tag="sSn")
            EC = bigp.tile([128, T_ALL], f32, tag="EC")
            ES = bigp.tile([128, T_ALL], f32, tag="ES")
            for lo, wch in ((0, 512), (512, T_ALL - 512)):
                nc.scalar.activation(sC[:, lo:lo + wch], BT[:, lo:lo + wch],
                                     AF.Sin, bias=cPI2[:, 0:1])
                nc.scalar.activation(sSn[:, lo:lo + wch], BT[:, lo:lo + wch],
                                     AF.Sin)
                nc.vector.tensor_mul(EC[:, lo:lo + wch], sE[:, lo:lo + wch],
                                     sC[:, lo:lo + wch])
                nc.vector.tensor_mul(ES[:, lo:lo + wch], sE[:, lo:lo + wch],
                                     sSn[:, lo:lo + wch])

            # ---- one level of the sweep (shared by block and single-block
            # phases). V: children tile (P x wc) bf16; returns psum parents.
            def sweep_level(V, ECh, ESh, P, lT, lTs, lW):
                # returns pY (P x wc) = W @ (EC*Winv V + ES*WinvSwap V):
                # the per-edge transition matvec for every child, full width
                wc = V.shape[1]
                pU = psU.tile([P, wc], f32, tag="U")
                nc.tensor.matmul(pU[:], lT[:], V[:], start=True, stop=True)
                pUs = psUs.tile([P, wc], f32, tag="Us")
                nc.tensor.matmul(pUs[:], lTs[:], V[:], start=True, stop=True)
                m1 = sb.tile([P, wc], bf16, tag="m1")
                nc.vector.tensor_mul(m1[:], ECh, pU[:])
                m2 = sb.tile([P, wc], bf16, tag="m2")
                nc.vector.tensor_mul(m2[:], ESh, pUs[:])
                pY = psYe.tile([P, wc], f32, tag="Ye")
                nc.tensor.matmul(pY[:], lW[:], m1[:], start=True, stop=False)
                nc.tensor.matmul(pY[:], lW[:], m2[:], start=False, stop=True)
                return pY

            def combine(pY, P, wp, out_dt):
                # parent = Y_even * (g * Y_odd); one DVE copy + one fused
                # scalar_tensor_tensor (single-PSUM-operand rule)
                sYe = sb.tile([P, wp], f32, tag="sYe")
                nc.vector.tensor_copy(sYe[:], pY[:, 0::2])
                Vn = sb.tile([P, wp], out_dt, tag="Vc")
                nc.vector.scalar_tensor_tensor(
                    Vn[:], pY[:, 1::2], c_gcol[0:P, 0:1], sYe[:],
                    mybir.AluOpType.mult, mybir.AluOpType.mult)
                return Vn

            # ---- phase 3: block sweep, levels 1..9
            V = sX
            lsW = None  # (8 x wp) per-parent log-scales, tracked from h=4
            for h in range(1, 10):
                wc = BLK_W[h - 1]
                lo = int(BLK_OFF[h - 1])
                wp = wc // 2
                pY = sweep_level(
                    V[:], EC[:, lo:lo + wc], ES[:, lo:lo + wc], 128,
                    c_winvT, c_wswapT, c_wT)
                if h in NORM_LEVELS:
                    praw = combine(pY, 128, wp, bf16)
                    pSb = psYo.tile([128, wp], f32, tag="Yo")
                    nc.tensor.matmul(pSb[:], c_onesbd[:], praw[:],
                                     start=True, stop=True)
                    pSc = psUs.tile([8, wp], f32, tag="Us")
                    nc.tensor.matmul(pSc[:], c_onesc[:], praw[:],
                                     start=True, stop=True)
                    rb = sb.tile([128, wp], f32, tag="rb")
                    nc.vector.reciprocal(rb[:], pSb[:])
                    Vn = sb.tile([128, wp], bf16, tag="V")
                    nc.vector.tensor_mul(Vn[:], praw[:], rb[:])
                    sSc = sb.tile([8, wp], f32, tag="sSc")
                    nc.vector.tensor_copy(sSc[:], pSc[:])
                    lnS = sb.tile([8, wp], f32, tag="lnS")
                    nc.scalar.activation(lnS[:], sSc[:], AF.Ln)
                    if lsW is None:
                        lsW = lnS
                    else:
                        ls2 = sb.tile([8, wp], f32, tag="ls")
                        nc.gpsimd.tensor_add(ls2[:], lsW[:, 0::2], lsW[:, 1::2])
                        ls3 = sb.tile([8, wp], f32, tag="ls")
                        nc.gpsimd.tensor_add(ls3[:], ls2[:], lnS[:])
                        lsW = ls3
                else:
                    # h=9's output is only consumed as a per-partition
                    # scalar (phase 4), which must be f32
                    Vn = combine(pY, 128, wp, f32 if h == 9 else bf16)
                    if lsW is not None:
                        ls2 = sb.tile([8, wp], f32, tag="ls")
                        nc.gpsimd.tensor_add(ls2[:], lsW[:, 0::2], lsW[:, 1::2])
                        lsW = ls2
                V = Vn

            # ---- phase 4: reshape core state to single block
            # V (128 x 1) -> (16 x 8); ls (8 x 1) -> (1 x 8)
            rhs8 = sb.tile([128, 8], bf16, tag="rhs8")
            nc.vector.tensor_scalar_mul(rhs8[:], c_onesc[:], V[:, 0:1])
            pV9 = psU.tile([16, 8], f32, tag="U")
            nc.tensor.matmul(pV9[:], c_itile[:], rhs8[:], start=True, stop=True)
            sV = sb.tile([16, 8], bf16, tag="sV")
            nc.vector.tensor_copy(sV[:], pV9[:])
            pls = psUs.tile([1, 8], f32, tag="Us")
            nc.tensor.matmul(pls[:], lsW[:], c_id8f[:], start=True, stop=True)
            sls = sb.tile([1, 8], f32, tag="sls")
            nc.vector.tensor_copy(sls[:], pls[:])

            w1 = c_winvT[0:16, 0:16]
            w1s = c_wswapT[0:16, 0:16]
            wW = c_wT[0:16, 0:16]

            # ---- levels 10..12 on a single block (top factors live in
            # block 0 of EC/ES at column offset TOPO)
            topoff = TOPO
            n = 8
            for h in (10, 11, 12):
                pY = sweep_level(
                    sV[:], EC[0:16, topoff:topoff + n],
                    ES[0:16, topoff:topoff + n], 16, w1, w1s, wW)
                topoff += n
                n //= 2
                # level 12's vector travels through the AllGather raw (f32);
                # no normalization needed -- drift since the h=8 norm stays
                # comfortably inside f32/Ln range and is absorbed by the
                # final log
                sV = combine(pY, 16, n, f32 if n == 1 else bf16)
                sls2 = sb.tile([1, n], f32, tag="sls")
                nc.gpsimd.tensor_add(sls2[:], sls[:, 0::2], sls[:, 1::2])
                sls = sls2

            ls12 = sb.tile([16, 1], f32, tag="ls12")
            nc.vector.memset(ls12[:], 0.0)
            nc.vector.tensor_copy(ls12[0:1, :], sls[:])

            # ---- AllGather of (16-vec, logscale) across the 8 cores
            nc.sync.dma_start(agin[0:16, 0:1], sV[:])
            nc.sync.dma_start(agin[16:32, 0:1], ls12[:])
            nc.gpsimd.collective_compute(
                "AllGather",
                OP.bypass,
                replica_groups=[list(range(NCORES))],
                ins=[agin[:, :].opt()],
                outs=[agout[:, :].opt()],
            )
            ag2 = agout[:, 0].rearrange("(r v) -> v r", v=AGW)
            gv0 = sb.tile([16, 8], f32, tag="gv0")
            nc.sync.dma_start(gv0[:], ag2[0:16, :])
            gv = sb.tile([16, 8], bf16, tag="gv")
            nc.vector.tensor_copy(gv[:], gv0[:])
            gls = sb.tile([1, 8], f32, tag="gls")
            nc.sync.dma_start(gls[:], ag2[16:17, :])
            tot0 = sb.tile([1, 1], f32, tag="tot0")
            nc.vector.tensor_reduce(tot0[:], gls[:], mybir.AxisListType.X,
                                    OP.add)
            tot = sb.tile([1, 1], f32, tag="tot")
            nc.vector.tensor_scalar_add(tot[:], tot0[:], float(-CORR))

            # ---- levels 13..16 (replicated on every core)
            sV = gv
            n = 8
            for h in (13, 14, 15):
                pY = sweep_level(
                    sV[:], EC[0:16, topoff:topoff + n],
                    ES[0:16, topoff:topoff + n], 16, w1, w1s, wW)
                topoff += n
                n //= 2
                sV = combine(pY, 16, n, bf16)
            # root: unifurcating, left child only, no growth
            pY = sweep_level(
                sV[:], EC[0:16, topoff:topoff + 1], ES[0:16, topoff:topoff + 1],
                16, w1, w1s, wW)

            lnv = sb.tile([16, 1], f32, tag="lnv")
            nc.scalar.activation(lnv[:], pY[:], AF.Ln)
            ptb = psUs.tile([16, 1], f32, tag="Us")
            nc.tensor.matmul(ptb[:], c_onesf[:], tot[:], start=True, stop=True)
            outv = sb.tile([16, 1], f32, tag="outv")
            nc.vector.tensor_add(outv[:], lnv[:], ptb[:])
            nc.sync.dma_start(out[:, :], outv[:])

    if split_waits:
        _split_multi_waits(nc)
    return nc


def _host_prep(branch_lens, init_partials, Q, growth_rates):
    bl = np.ascontiguousarray(np.asarray(branch_lens, dtype=F32))
    ip = np.ascontiguousarray(np.asarray(init_partials, dtype=F32))
    Q64 = np.asarray(Q, dtype=np.float64)
    g64 = np.asarray(growth_rates, dtype=np.float64)
    R = Q64 - np.diag(g64)
    Wr, Winv, a, bsig, swap = _real_eig(R)

    I8 = np.eye(8)

    def bf(x):
        return np.ascontiguousarray(
            np.asarray(x, dtype=np.float32).astype(BF16))

    consts = {
        "winvT_bd": bf(np.kron(I8, Winv.T)),
        "wswapT_bd": bf(np.kron(I8, Winv[swap, :].T)),
        "wT_bd": bf(np.kron(I8, Wr.T)),
        "ones_bd": bf(np.kron(I8, np.ones((S, S)))),
        "onesc": bf(np.kron(I8, np.ones((S, 1)))),
        "itile": bf(np.tile(np.eye(S), (8, 1))),
        "ident": np.ascontiguousarray(np.eye(128), dtype=F32),
        "expd": np.ascontiguousarray(np.kron(I8, np.ones((1, S))), dtype=F32),
        "avec": np.ascontiguousarray(np.tile(a, 8)[:, None], dtype=F32),
        "bvec": np.ascontiguousarray(np.tile(bsig, 8)[:, None], dtype=F32),
        "onesf": np.ones((1, 16), dtype=F32),
        "gcol": np.ascontiguousarray(np.tile(g64, 8)[:, None], dtype=F32),
        "id8f": np.ascontiguousarray(np.eye(8), dtype=F32),
    }

    in_maps = []
    for c in range(NCORES):
        # leaf tile: A[p, 128k+16u+cc] = ip[c*4096 + 512u + 128k + p, cc]
        ipc = ip[c * LPC:(c + 1) * LPC].reshape(8, 4, 128, S)  # [u, k, p, cc]
        a_leaf = np.ascontiguousarray(
            ipc.transpose(2, 1, 0, 3).reshape(128, 512), dtype=F32)

        t_blk = np.zeros((8, T_ALL), dtype=F32)
        for hc in range(9):
            w = LPB >> hc
            base = OFFS[hc] + c * (LPC >> hc)
            seg = bl[base: base + (LPC >> hc)].reshape(8, w)
            t_blk[:, int(BLK_OFF[hc]): int(BLK_OFF[hc]) + w] = seg
        # top edges into block-0 columns TOPO..TOPO+28
        tt = np.concatenate([
            bl[OFFS[9] + c * 8: OFFS[9] + c * 8 + 8],
            bl[OFFS[10] + c * 4: OFFS[10] + c * 4 + 4],
            bl[OFFS[11] + c * 2: OFFS[11] + c * 2 + 2],
            bl[OFFS[12]: OFFS[12] + 8],
            bl[OFFS[13]: OFFS[13] + 4],
            bl[OFFS[14]: OFFS[14] + 2],
            bl[OFFS[15]: OFFS[15] + 1],
        ])
        t_blk[0, TOPO:TOPO + 29] = tt

        in_maps.append({"a_leaf": a_leaf, "t_blk": t_blk, **consts})
    return in_maps


def kernel(postorder, children, parents, branch_lens, init_partials, Q,
           levels, growth_rates, *, _trace=False):
    in_maps = _host_prep(branch_lens, init_partials, Q, growth_rates)
    nc = build_nc()
    res = run_bass_kernel_spmd(nc, in_maps, core_ids=list(range(NCORES)),
                               trace=_trace)
    out = np.asarray(res.results[0]["out"], dtype=F32).reshape(S)
    if _trace:
        kernel.last_exec_time_ns = res.exec_time_ns
        kernel.last_results = res
    return out


# revision 23
# speedup vs baseline: 1.1622x; 1.1622x over previous
"""Trainium2 Bass kernel for Felsenstein pruning on a perfect binary tree
(nn_BaseLikelihoodModel: batched expm over edges + level-synchronous sweep).

Key algorithmic idea: every edge matrix is expm(t_e * R) for ONE shared
16x16 rate matrix R = Q - diag(growth).  A real block-eigendecomposition
R = W M W^-1 (computed host-side in f64, O(16^3)) turns the per-edge expm
into per-edge exp/sin factors plus shared 16x16 matmuls:

    expm(t R) v = W * B(t) * (W^-1 v)
    B(t) rows:   real eig k:    e^{a t} u_k
                 pair (k,k+1):  e^{a t}(cos(bt) u_k + sin(bt) u_{k+1})
                                e^{a t}(cos(bt) u_{k+1} - sin(bt) u_k)

The level sweep runs in probability space with per-node rescaling every 4
levels (accumulated per-node log-scales), mathematically identical to the
reference's log-space logsumexp recursion.

Sharding: 8 contiguous subtrees of 4096 leaves (one per core).  Within a
core, 8 partition-blocks of 16 states each hold 8 sub-subtrees of 512
leaves, giving full 128-partition utilization for levels 1..9.  Levels
10..12 run on a single 16-partition block.  One tiny AllGather (8 x 32 f32)
shares the per-core subtree roots + log-scales; levels 13..16 are computed
redundantly on every core.

Numerics: sweep matmuls run in bf16 (inputs) with f32 PSUM accumulation;
the per-node normalization is self-consistent (the same computed sum is
both divided out and logged), so quantization there cancels.  Edge factors
carry a constant boost e^BOOST to keep normalization sums inside the HW Ln
table's [2^-64, 2^64] domain; the exact total (N-1)*BOOST is subtracted at
the end.
"""
import math
import numpy as np
import ml_dtypes

import concourse.bass as bass
import concourse.mybir as mybir
import concourse.tile as tile
from concourse.bass_utils import run_bass_kernel_spmd

F32 = np.float32
BF16 = ml_dtypes.bfloat16
S = 16
L = 32768
N = 2 * L
NCORES = 8
NBLK = 8
LPC = L // NCORES          # 4096 leaves per core
LPB = LPC // NBLK          # 512 leaves per block
# block-sweep children widths per block for h=1..9
BLK_W = [LPB >> hc for hc in range(9)]            # 512..2
BLK_OFF = np.concatenate([[0], np.cumsum(BLK_W)])  # offsets into 1022
BLK_TOTAL = int(BLK_OFF[-1])                       # 1022
# top-edge factors live in block 0 of the same tiles, columns TOPO..TOPO+29
# layout: [lvl9 x8][lvl10 x4][lvl11 x2][lvl12 x8][lvl13 x4][lvl14 x2][lvl15 x1]
TOPO = BLK_TOTAL                                   # 1022
T_ALL = 1056                                       # padded total columns
CHUNKS = [(0, 512), (512, 512), (1024, 32)]
NORM_LEVELS = (4, 8)                               # + level 12 pre-gather

# global level offsets (node ids are level-contiguous, leaves first)
OFFS = [0]
for _h in range(1, 16):
    OFFS.append(OFFS[-1] + (L >> (_h - 1)))
ROOT = N - 1  # 65535; its only child is OFFS[15] = 65534

# Constant per-edge boost e^BOOST folded into the exp factors so the
# per-node normalization sums stay near 1.0 -- the HW Ln table only covers
# [2^-64, 2^64] (it clamps outside, which the simulator's exact np.log
# does not model).  The exact total, CORR, is subtracted at the end.
BOOST = 1.7
CORR = float(np.float64(N - 1) * np.float64(np.float32(BOOST)))
AGW = 32  # AllGather payload per core (16 state values + logscale + pad)


def _real_eig(R):
    """Real block eigendecomposition R = Wr @ M @ inv(Wr) with M block
    diagonal ([[a, b], [-b, a]] blocks for conjugate pairs).  Returns
    f64 Wr, Winv, a[k], bsig[k] (signed imag: +b row k, -b row k+1),
    swap[k] (pair permutation, identity on real rows)."""
    ev, V = np.linalg.eig(R)
    used = np.zeros(S, bool)
    order = np.argsort(-ev.real)
    cols = []
    for i in order:
        if used[i]:
            continue
        if abs(ev[i].imag) < 1e-12:
            cols.append(("real", i))
            used[i] = True
        else:
            j = None
            for i2 in order:
                if not used[i2] and i2 != i and abs(ev[i2] - ev[i].conj()) < 1e-8:
                    j = i2
                    break
            assert j is not None, "unpaired complex eigenvalue"
            ip = i if ev[i].imag > 0 else j
            cols.append(("pair", ip))
            used[i] = used[j] = True
    Wr = np.zeros((S, S))
    a = np.zeros(S)
    bsig = np.zeros(S)
    swap = np.arange(S)
    k = 0
    for c in cols:
        if c[0] == "real":
            i = c[1]
            Wr[:, k] = V[:, i].real
            a[k] = ev[i].real
            k += 1
        else:
            ip = c[1]
            lam = ev[ip]
            Wr[:, k] = V[:, ip].real
            Wr[:, k + 1] = V[:, ip].imag
            a[k] = a[k + 1] = lam.real
            bsig[k] = lam.imag
            bsig[k + 1] = -lam.imag
            swap[k] = k + 1
            swap[k + 1] = k
            k += 2
    assert k == S
    # column-normalize for conditioning; pairs share a scale to keep the
    # block structure valid
    scales = np.ones(S)
    kk = 0
    while kk < S:
        if swap[kk] == kk:
            scales[kk] = np.linalg.norm(Wr[:, kk])
            kk += 1
        else:
            s = math.sqrt(np.linalg.norm(Wr[:, kk]) * np.linalg.norm(Wr[:, kk + 1]))
            scales[kk] = scales[kk + 1] = s
            kk += 2
    Wr = Wr / scales[None, :]
    Winv = np.linalg.inv(Wr)
    return Wr, Winv, a, bsig, swap


def _split_multi_waits(nc):
    """Walrus codegen allows only ONE sync-wait slot per engine instruction
    (NEURON_ISA_TPB_EVENTS has a single wait field).  Tile's scheduler can
    attach several; move the extras onto prepended same-engine NoOps, which
    execute in queue order and therefore stall identically."""
    skip = (mybir.InstAllEngineBarrier, mybir.InstBranchHint,
            mybir.InstCompareAndBranch, mybir.InstUnconditionalBranch,
            mybir.InstIndirectBranch)
    for fn in nc.m.functions:
        for blk in fn.blocks:
            out = []
            for inst in blk.instructions:
                si = inst.sync_info
                if (si is not None and si.on_wait and len(si.on_wait) > 1
                        and not isinstance(inst, skip)):
                    waits = list(si.on_wait)
                    for i, w in enumerate(waits[:-1]):
                        nop = mybir.InstNoOp(
                            name=f"{inst.name}-wait{i}", ins=[], outs=[])
                        nop.engine = inst.engine
                        nop.sync_info = mybir.SyncInfo(
                            on_wait=[w], on_update=[])
                        out.append(nop)
                    inst.sync_info = mybir.SyncInfo(
                        on_wait=[waits[-1]], on_update=list(si.on_update or []))
                out.append(inst)
            blk.instructions = out


def build_nc(split_waits=True):
    f32 = mybir.dt.float32
    bf16 = mybir.dt.bfloat16
    AF = mybir.ActivationFunctionType
    OP = mybir.AluOpType
    nc = bass.Bass()

    # per-core inputs
    a_leaf = nc.dram_tensor("a_leaf", [128, 512], f32, kind="ExternalInput")
    t_blk = nc.dram_tensor("t_blk", [8, T_ALL], f32, kind="ExternalInput")
    # shared constants (bf16 for the sweep matmuls)
    winvT = nc.dram_tensor("winvT_bd", [128, 128], bf16, kind="ExternalInput")
    wswapT = nc.dram_tensor("wswapT_bd", [128, 128], bf16, kind="ExternalInput")
    wT = nc.dram_tensor("wT_bd", [128, 128], bf16, kind="ExternalInput")
    onesbd = nc.dram_tensor("ones_bd", [128, 128], bf16, kind="ExternalInput")
    onesc = nc.dram_tensor("onesc", [128, 8], bf16, kind="ExternalInput")
    itile = nc.dram_tensor("itile", [128, 16], bf16, kind="ExternalInput")
    winvTr = nc.dram_tensor("winvT_r", [128, 128], bf16, kind="ExternalInput")
    wswapTr = nc.dram_tensor("wswapT_r", [128, 128], bf16, kind="ExternalInput")
    wTr = nc.dram_tensor("wT_r", [128, 128], bf16, kind="ExternalInput")
    ident = nc.dram_tensor("ident", [128, 128], f32, kind="ExternalInput")
    expd = nc.dram_tensor("expd", [8, 128], f32, kind="ExternalInput")
    avec = nc.dram_tensor("avec", [128, 1], f32, kind="ExternalInput")
    bvec = nc.dram_tensor("bvec", [128, 1], f32, kind="ExternalInput")
    onesf = nc.dram_tensor("onesf", [1, 16], f32, kind="ExternalInput")
    gcol = nc.dram_tensor("gcol", [128, 1], f32, kind="ExternalInput")
    id8f = nc.dram_tensor("id8f", [8, 8], f32, kind="ExternalInput")
    out = nc.dram_tensor("out", [16, 1], f32, kind="ExternalOutput")
    # collective bounce buffers (internal DRAM)
    agin = nc.dram_tensor("agin", [AGW, 1], f32)
    agout = nc.dram_tensor("agout", [NCORES * AGW, 1], f32, addr_space="Shared")

    PI2 = float(np.pi / 2)

    with tile.TileContext(nc) as tc:
        with (
            tc.tile_pool(name="const", bufs=1) as cp,
            tc.tile_pool(name="sb", bufs=2) as sb,
            tc.tile_pool(name="big", bufs=1) as bigp,
            tc.tile_pool(name="psT", bufs=2, space="PSUM") as psT,
            tc.tile_pool(name="psU", bufs=1, space="PSUM") as psU,
            tc.tile_pool(name="psUs", bufs=1, space="PSUM") as psUs,
            tc.tile_pool(name="psYe", bufs=2, space="PSUM") as psYe,
            tc.tile_pool(name="psYo", bufs=2, space="PSUM") as psYo,
        ):
            # ---- constants into SBUF (order matters: early ones feed the
            # first compute)
            def cload(dram, shape, dt, tag):
                t = cp.tile(shape, dt, tag=tag)
                nc.sync.dma_start(t[:], dram[:, :])
                return t

            sTb = cp.tile([8, T_ALL], f32, tag="tblk")
            nc.sync.dma_start(sTb[:], t_blk[:, :])
            c_expd = cload(expd, [8, 128], f32, "expd")
            c_avec = cload(avec, [128, 1], f32, "avec")
            c_bvec = cload(bvec, [128, 1], f32, "bvec")
            sA = bigp.tile([128, 512], f32, tag="aleaf")
            nc.sync.dma_start(sA[:], a_leaf[:, :])
            c_ident = cload(ident, [128, 128], f32, "ident")
            c_winvT = cload(winvT, [128, 128], bf16, "winvT")
            c_wswapT = cload(wswapT, [128, 128], bf16, "wswapT")
            c_wT = cload(wT, [128, 128], bf16, "wT")
            c_onesbd = cload(onesbd, [128, 128], bf16, "onesbd")
            c_onesc = cload(onesc, [128, 8], bf16, "onesc")
            c_itile = cload(itile, [128, 16], bf16, "itile")
            c_winvTr = cload(winvTr, [128, 128], bf16, "winvTr")
            c_wswapTr = cload(wswapTr, [128, 128], bf16, "wswapTr")
            c_wTr = cload(wTr, [128, 128], bf16, "wTr")
            c_onesf = cload(onesf, [1, 16], f32, "onesf")
            c_id8f = cload(id8f, [8, 8], f32, "id8f")
            c_gcol = cload(gcol, [128, 1], f32, "gcol")
            cPI2 = cp.tile([128, 1], f32, tag="pi2")
            nc.vector.memset(cPI2[:], PI2)
            cBOOST = cp.tile([128, 1], f32, tag="boost")
            nc.vector.memset(cBOOST[:], float(BOOST))

            # ---- phase 1: leaf one-hots (exp of log one-hots) + transpose
            # into blocked layout X[16u+c, 128k+j] = onehot[leaf 512u+128k+j][c]
            sAe = bigp.tile([128, 512], f32, tag="aexp")
            nc.scalar.activation(sAe[:], sA[:], AF.Exp)
            sX = bigp.tile([128, 512], bf16, tag="V0")
            for k in range(4):
                pT = psT.tile([128, 128], f32, tag="T")
                nc.tensor.transpose(pT[:], sAe[:, 128 * k:128 * (k + 1)],
                                    c_ident[:])
                nc.vector.tensor_copy(sX[:, 128 * k:128 * (k + 1)], pT[:])

            # ---- phase 2: edge factors EC = e^{at+BOOST}cos(bt),
            # ES = e^{at+BOOST}sin(bt) for all edges (block sweep cols
            # 0..1021, top edges in block 0 cols 1022..1050).
            # T128 = t broadcast down each 16-partition block (one matmul),
            # then AT/BT via per-partition scalar mults straight to SBUF;
            # activations batched by table set (exp first, then sin).
            AT = bigp.tile([128, T_ALL], f32, tag="AT")
            BT = bigp.tile([128, T_ALL], f32, tag="BT")
            for lo, wch in CHUNKS:
                pT128 = psT.tile([128, wch], f32, tag="T")
                nc.tensor.matmul(pT128[:], c_expd[:], sTb[:, lo:lo + wch],
                                 start=True, stop=True)
                nc.vector.tensor_scalar_mul(AT[:, lo:lo + wch], pT128[:],
                                            c_avec[:, 0:1])
                nc.vector.tensor_scalar_mul(BT[:, lo:lo + wch], pT128[:],
                                            c_bvec[:, 0:1])
            # exp/sins and the EC/ES multiplies chunked so the columns
            # level 1 needs are ready before the tail is processed; exps all
            # precede sins so each ACT table set loads exactly once
            # ---- PE queue-observer preamble: one tiny matmul per DMA'd
            # matmul operand so later matmuls never need two DMA-queue waits
            # (walrus allows a single sync wait per PE instruction).
            obs_bf = (c_winvT, c_wswapT, c_wT, c_onesbd, c_onesc,
                      c_itile, c_winvTr, c_wswapTr, c_wTr)
            pobs = psYe.tile([1, 1], f32, tag="Ye")
            for i, o in enumerate(obs_bf):
                nc.tensor.matmul(pobs[:], o[0:1, 0:1], o[0:1, 0:1],
                                 start=(i == 0), stop=(i == len(obs_bf) - 1))
            obs_f = (c_ident, c_expd, sTb, c_onesf, c_id8f)
            pobs2 = psYo.tile([1, 1], f32, tag="Yo")
            for i, o in enumerate(obs_f):
                nc.tensor.matmul(pobs2[:], o[0:1, 0:1], o[0:1, 0:1],
                                 start=(i == 0), stop=(i == len(obs_f) - 1))

            sE = bigp.tile([128, T_ALL], f32, tag="sE")
            for lo, wch in ((0, 512), (512, T_ALL - 512)):
                nc.scalar.activation(sE[:, lo:lo + wch], AT[:, lo:lo + wch],
                                     AF.Exp, bias=cBOOST[:, 0:1])
            sC = bigp.tile([128, T_ALL], f32, tag="sC")
            sSn = bigp.tile([128, T_ALL], f32, tag="sSn")
            EC = bigp.tile([128, T_ALL], f32, tag="EC")
            ES = bigp.tile([128, T_ALL], f32, tag="ES")
            for lo, wch in ((0, 512), (512, T_ALL - 512)):
                nc.scalar.activation(sC[:, lo:lo + wch], BT[:, lo:lo + wch],
                                     AF.Sin, bias=cPI2[:, 0:1])
                nc.scalar.activation(sSn[:, lo:lo + wch], BT[:, lo:lo + wch],
                                     AF.Sin)
                nc.vector.tensor_mul(EC[:, lo:lo + wch], sE[:, lo:lo + wch],
                                     sC[:, lo:lo + wch])
                nc.vector.tensor_mul(ES[:, lo:lo + wch], sE[:, lo:lo + wch],
                                     sSn[:, lo:lo + wch])

            # ---- one level of the sweep (shared by block and single-block
            # phases). V: children tile (P x wc) bf16; returns psum parents.
            def sweep_level(V, ECh, ESh, P, lT, lTr, lTs, lTsr, lW, lWr):
                # returns pY (P x wc) = W @ (EC*Winv V + ES*WinvSwap V):
                # the per-edge transition matvec for every child, full width.
                # Each constant is applied as base_bf16 + residual_bf16
                # (PSUM-accumulated), recovering ~f32 constant precision --
                # plain bf16 constants shift every edge's log-factor the
                # same way and the bias sums over all 65535 edges.
                wc = V.shape[1]
                pU = psU.tile([P, wc], f32, tag="U")
                nc.tensor.matmul(pU[:], lT[:], V[:], start=True, stop=False)
                nc.tensor.matmul(pU[:], lTr[:], V[:], start=False, stop=True)
                pUs = psUs.tile([P, wc], f32, tag="Us")
                nc.tensor.matmul(pUs[:], lTs[:], V[:], start=True, stop=False)
                nc.tensor.matmul(pUs[:], lTsr[:], V[:], start=False, stop=True)
                m1 = sb.tile([P, wc], bf16, tag="m1")
                nc.vector.tensor_mul(m1[:], ECh, pU[:])
                m2 = sb.tile([P, wc], bf16, tag="m2")
                nc.vector.tensor_mul(m2[:], ESh, pUs[:])
                pY = psYe.tile([P, wc], f32, tag="Ye")
                nc.tensor.matmul(pY[:], lW[:], m1[:], start=True, stop=False)
                nc.tensor.matmul(pY[:], lW[:], m2[:], start=False, stop=False)
                nc.tensor.matmul(pY[:], lWr[:], m1[:], start=False, stop=False)
                nc.tensor.matmul(pY[:], lWr[:], m2[:], start=False, stop=True)
                return pY

            def combine(pY, P, wp, out_dt):
                # parent = Y_even * (g * Y_odd); one DVE copy + one fused
                # scalar_tensor_tensor (single-PSUM-operand rule)
                sYe = sb.tile([P, wp], f32, tag="sYe")
                nc.vector.tensor_copy(sYe[:], pY[:, 0::2])
                Vn = sb.tile([P, wp], out_dt, tag="Vc")
                nc.vector.scalar_tensor_tensor(
                    Vn[:], pY[:, 1::2], c_gcol[0:P, 0:1], sYe[:],
                    mybir.AluOpType.mult, mybir.AluOpType.mult)
                return Vn

            # ---- phase 3: block sweep, levels 1..9
            V = sX
            lsW = None  # (8 x wp) per-parent log-scales, tracked from h=4
            for h in range(1, 10):
                wc = BLK_W[h - 1]
                lo = int(BLK_OFF[h - 1])
                wp = wc // 2
                pY = sweep_level(
                    V[:], EC[:, lo:lo + wc], ES[:, lo:lo + wc], 128,
                    c_winvT, c_winvTr, c_wswapT, c_wswapTr, c_wT, c_wTr)
                if h in NORM_LEVELS:
                    praw = combine(pY, 128, wp, bf16)
                    pSb = psYo.tile([128, wp], f32, tag="Yo")
                    nc.tensor.matmul(pSb[:], c_onesbd[:], praw[:],
                                     start=True, stop=True)
                    pSc = psUs.tile([8, wp], f32, tag="Us")
                    nc.tensor.matmul(pSc[:], c_onesc[:], praw[:],
                                     start=True, stop=True)
                    rb = sb.tile([128, wp], f32, tag="rb")
                    nc.vector.reciprocal(rb[:], pSb[:])
                    Vn = sb.tile([128, wp], bf16, tag="V")
                    nc.vector.tensor_mul(Vn[:], praw[:], rb[:])
                    sSc = sb.tile([8, wp], f32, tag="sSc")
                    nc.vector.tensor_copy(sSc[:], pSc[:])
                    lnS = sb.tile([8, wp], f32, tag="lnS")
                    nc.scalar.activation(lnS[:], sSc[:], AF.Ln)
                    if lsW is None:
                        lsW = lnS
                    else:
                        ls2 = sb.tile([8, wp], f32, tag="ls")
                        nc.gpsimd.tensor_add(ls2[:], lsW[:, 0::2], lsW[:, 1::2])
                        ls3 = sb.tile([8, wp], f32, tag="ls")
                        nc.gpsimd.tensor_add(ls3[:], ls2[:], lnS[:])
                        lsW = ls3
                else:
                    # h=9's output is only consumed as a per-partition
                    # scalar (phase 4), which must be f32
                    Vn = combine(pY, 128, wp, f32 if h == 9 else bf16)
                    if lsW is not None:
                        ls2 = sb.tile([8, wp], f32, tag="ls")
                        nc.gpsimd.tensor_add(ls2[:], lsW[:, 0::2], lsW[:, 1::2])
                        lsW = ls2
                V = Vn

            # ---- phase 4: reshape core state to single block
            # V (128 x 1) -> (16 x 8); ls (8 x 1) -> (1 x 8)
            rhs8 = sb.tile([128, 8], bf16, tag="rhs8")
            nc.vector.tensor_scalar_mul(rhs8[:], c_onesc[:], V[:, 0:1])
            pV9 = psU.tile([16, 8], f32, tag="U")
            nc.tensor.matmul(pV9[:], c_itile[:], rhs8[:], start=True, stop=True)
            sV = sb.tile([16, 8], bf16, tag="sV")
            nc.vector.tensor_copy(sV[:], pV9[:])
            pls = psUs.tile([1, 8], f32, tag="Us")
            nc.tensor.matmul(pls[:], lsW[:], c_id8f[:], start=True, stop=True)
            sls = sb.tile([1, 8], f32, tag="sls")
            nc.vector.tensor_copy(sls[:], pls[:])

            w1 = c_winvT[0:16, 0:16]
            w1r = c_winvTr[0:16, 0:16]
            w1s = c_wswapT[0:16, 0:16]
            w1sr = c_wswapTr[0:16, 0:16]
            wW = c_wT[0:16, 0:16]
            wWr = c_wTr[0:16, 0:16]

            # ---- levels 10..12 on a single block (top factors live in
            # block 0 of EC/ES at column offset TOPO)
            topoff = TOPO
            n = 8
            for h in (10, 11, 12):
                pY = sweep_level(
                    sV[:], EC[0:16, topoff:topoff + n],
                    ES[0:16, topoff:topoff + n], 16,
                    w1, w1r, w1s, w1sr, wW, wWr)
                topoff += n
                n //= 2
                # level 12's vector travels through the AllGather raw (f32);
                # no normalization needed -- drift since the h=8 norm stays
                # comfortably inside f32/Ln range and is absorbed by the
                # final log
                sV = combine(pY, 16, n, f32 if n == 1 else bf16)
                sls2 = sb.tile([1, n], f32, tag="sls")
                nc.gpsimd.tensor_add(sls2[:], sls[:, 0::2], sls[:, 1::2])
                sls = sls2

            ls12 = sb.tile([16, 1], f32, tag="ls12")
            nc.vector.memset(ls12[:], 0.0)
            nc.vector.tensor_copy(ls12[0:1, :], sls[:])

            # ---- AllGather of (16-vec, logscale) across the 8 cores
            nc.sync.dma_start(agin[0:16, 0:1], sV[:])
            nc.sync.dma_start(agin[16:32, 0:1], ls12[:])
            nc.gpsimd.collective_compute(
                "AllGather",
                OP.bypass,
                replica_groups=[list(range(NCORES))],
                ins=[agin[:, :].opt()],
                outs=[agout[:, :].opt()],
            )
            ag2 = agout[:, 0].rearrange("(r v) -> v r", v=AGW)
            gv0 = sb.tile([16, 8], f32, tag="gv0")
            nc.sync.dma_start(gv0[:], ag2[0:16, :])
            gv = sb.tile([16, 8], bf16, tag="gv")
            nc.vector.tensor_copy(gv[:], gv0[:])
            gls = sb.tile([1, 8], f32, tag="gls")
            nc.sync.dma_start(gls[:], ag2[16:17, :])
            tot0 = sb.tile([1, 1], f32, tag="tot0")
            nc.vector.tensor_reduce(tot0[:], gls[:], mybir.AxisListType.X,
                                    OP.add)
            tot = sb.tile([1, 1], f32, tag="tot")
            nc.vector.tensor_scalar_add(tot[:], tot0[:], float(-CORR))

            # ---- levels 13..16 (replicated on every core)
            sV = gv
            n = 8
            for h in (13, 14, 15):
                pY = sweep_level(
                    sV[:], EC[0:16, topoff:topoff + n],
                    ES[0:16, topoff:topoff + n], 16,
                    w1, w1r, w1s, w1sr, wW, wWr)
                topoff += n
                n //= 2
                sV = combine(pY, 16, n, bf16)
            # root: unifurcating, left child only, no growth
            pY = sweep_level(
                sV[:], EC[0:16, topoff:topoff + 1], ES[0:16, topoff:topoff + 1],
                16, w1, w1r, w1s, w1sr, wW, wWr)

            lnv = sb.tile([16, 1], f32, tag="lnv")
            nc.scalar.activation(lnv[:], pY[:], AF.Ln)
            ptb = psUs.tile([16, 1], f32, tag="Us")
            nc.tensor.matmul(ptb[:], c_onesf[:], tot[:], start=True, stop=True)
            outv = sb.tile([16, 1], f32, tag="outv")
            nc.vector.tensor_add(outv[:], lnv[:], ptb[:])
            nc.sync.dma_start(out[:, :], outv[:])

    if split_waits:
        _split_multi_waits(nc)
    return nc


def _host_prep(branch_lens, init_partials, Q, growth_rates):
    bl = np.ascontiguousarray(np.asarray(branch_lens, dtype=F32))
    ip = np.ascontiguousarray(np.asarray(init_partials, dtype=F32))
    Q64 = np.asarray(Q, dtype=np.float64)
    g64 = np.asarray(growth_rates, dtype=np.float64)
    R = Q64 - np.diag(g64)
    Wr, Winv, a, bsig, swap = _real_eig(R)

    I8 = np.eye(8)

    def bf(x):
        return np.ascontiguousarray(
            np.asarray(x, dtype=np.float32).astype(BF16))

    def bfres(x):
        x32 = np.asarray(x, dtype=np.float32)
        return bf(x32 - x32.astype(BF16).astype(np.float32))

    consts = {
        "winvT_bd": bf(np.kron(I8, Winv.T)),
        "wswapT_bd": bf(np.kron(I8, Winv[swap, :].T)),
        "wT_bd": bf(np.kron(I8, Wr.T)),
        "ones_bd": bf(np.kron(I8, np.ones((S, S)))),
        "onesc": bf(np.kron(I8, np.ones((S, 1)))),
        "itile": bf(np.tile(np.eye(S), (8, 1))),
        "winvT_r": bfres(np.kron(I8, Winv.T)),
        "wswapT_r": bfres(np.kron(I8, Winv[swap, :].T)),
        "wT_r": bfres(np.kron(I8, Wr.T)),
        "ident": np.ascontiguousarray(np.eye(128), dtype=F32),
        "expd": np.ascontiguousarray(np.kron(I8, np.ones((1, S))), dtype=F32),
        "avec": np.ascontiguousarray(np.tile(a, 8)[:, None], dtype=F32),
        "bvec": np.ascontiguousarray(np.tile(bsig, 8)[:, None], dtype=F32),
        "onesf": np.ones((1, 16), dtype=F32),
        "gcol": np.ascontiguousarray(np.tile(g64, 8)[:, None], dtype=F32),
        "id8f": np.ascontiguousarray(np.eye(8), dtype=F32),
    }

    in_maps = []
    for c in range(NCORES):
        # leaf tile: A[p, 128k+16u+cc] = ip[c*4096 + 512u + 128k + p, cc]
        ipc = ip[c * LPC:(c + 1) * LPC].reshape(8, 4, 128, S)  # [u, k, p, cc]
        a_leaf = np.ascontiguousarray(
            ipc.transpose(2, 1, 0, 3).reshape(128, 512), dtype=F32)

        t_blk = np.zeros((8, T_ALL), dtype=F32)
        for hc in range(9):
            w = LPB >> hc
            base = OFFS[hc] + c * (LPC >> hc)
            seg = bl[base: base + (LPC >> hc)].reshape(8, w)
            t_blk[:, int(BLK_OFF[hc]): int(BLK_OFF[hc]) + w] = seg
        # top edges into block-0 columns TOPO..TOPO+28
        tt = np.concatenate([
            bl[OFFS[9] + c * 8: OFFS[9] + c * 8 + 8],
            bl[OFFS[10] + c * 4: OFFS[10] + c * 4 + 4],
            bl[OFFS[11] + c * 2: OFFS[11] + c * 2 + 2],
            bl[OFFS[12]: OFFS[12] + 8],
            bl[OFFS[13]: OFFS[13] + 4],
            bl[OFFS[14]: OFFS[14] + 2],
            bl[OFFS[15]: OFFS[15] + 1],
        ])
        t_blk[0, TOPO:TOPO + 29] = tt

        in_maps.append({"a_leaf": a_leaf, "t_blk": t_blk, **consts})
    return in_maps


def kernel(postorder, children, parents, branch_lens, init_partials, Q,
           levels, growth_rates, *, _trace=False):
    in_maps = _host_prep(branch_lens, init_partials, Q, growth_rates)
    nc = build_nc()
    res = run_bass_kernel_spmd(nc, in_maps, core_ids=list(range(NCORES)),
                               trace=_trace)
    out = np.asarray(res.results[0]["out"], dtype=F32).reshape(S)
    if _trace:
        kernel.last_exec_time_ns = res.exec_time_ns
        kernel.last_results = res
    return out
